# revision 4
# baseline (speedup 1.0000x reference)
"""GCN classifier (2x GCNConv + mean-pool + linear head) on 8 Trainium2
NeuronCores via Bass/Tile.

Sharding: nodes (and their incident edges, grouped by destination) are
partitioned across the 8 cores; the small weight matrices are replicated;
per-layer node features are AllGathered into a full bf16 table that each
core gathers source rows from; the per-graph mean-pool partial sums are
AllReduced.

GCN normalization is factored as out = dinv * (A @ (dinv * (x @ W))) with
dinv = 1/sqrt(deg+selfloop), so the edge aggregation itself needs no
per-edge scaling: each 128-destination-node tile accumulates
S_chunk^T @ G_chunk in PSUM, where G_chunk is 128 dma_gather'ed source
rows (bf16) and S_chunk is a one-hot selection matrix generated on the
vector engine via iota == dst_local.
"""

import math
from dataclasses import dataclass, field

import ml_dtypes
import numpy as np

import concourse.bacc as bacc
import concourse.mybir as mybir
import concourse.tile as tile
from concourse.bass_utils import run_bass_kernel_spmd

P = 128
NCORES = 8
NWIN = 4  # int16 gather index windows over the node table

BF16 = mybir.dt.bfloat16
F32 = mybir.dt.float32
I16 = mybir.dt.int16


@dataclass
class Dims:
    n_nodes: int = 100000
    n_graphs: int = 512
    kin: int = 384
    hid: int = 128
    out: int = 2
    st: int = 7  # dst tiles per gather super-tile

    @property
    def shard_real(self):
        return self.n_nodes // NCORES

    @property
    def shard(self):  # padded per-core shard (multiple of 128)
        return ((self.shard_real + P - 1) // P) * P

    @property
    def nt(self):  # dst tiles per core
        return self.shard // P

    @property
    def npad(self):
        return self.shard * NCORES

    @property
    def ws(self):  # gather window rows
        assert self.npad % NWIN == 0
        return self.npad // NWIN


@dataclass
class Plan:
    dims: Dims
    K: np.ndarray          # [nt, NWIN] chunks per (tile, window), maxed over cores
    chunk_base: np.ndarray  # [nt, NWIN] first global chunk id of (t, w)
    nch: int               # total chunks per conv
    icols: np.ndarray      # [n_st, NWIN] idx cols per gather call
    icol_base: np.ndarray  # [n_st, NWIN] first idx col of call
    n_st: int
    st_tiles: list         # list of tile-index lists per super-tile
    # per-core input arrays
    eidx: list = field(default_factory=list)   # int16 [128, ICOLS]
    eld: list = field(default_factory=list)    # f32 [128, nch]
    dinv2d: list = field(default_factory=list)  # f32 [128, nt]
    bq: list = field(default_factory=list)     # bf16 [1, shard]
    batchval: list = field(default_factory=list)  # f32 [128, nt]
    cnt_inv: np.ndarray = None  # f32 [128, ng//128]
    dinv_full: np.ndarray = None  # f32 [n_nodes]


def _wrap16(idx_flat, ncols):
    """Flat int16 index list -> [128, ncols] wrapped layout (16-partition
    wrap, replicated over the 8 q7 cores)."""
    arr = np.zeros((16, ncols), dtype=np.int16)
    n = len(idx_flat)
    arr.reshape(-1, order="F").flat[:n] = idx_flat  # i -> [i%16, i//16]
    return np.tile(arr, (8, 1))


def make_plan(edge_index, batch, dims: Dims) -> Plan:
    d = dims
    SR, SH, NT, WS = d.shard_real, d.shard, d.nt, d.ws
    n = d.n_nodes

    src = np.asarray(edge_index[0], dtype=np.int64)
    dst = np.asarray(edge_index[1], dtype=np.int64)
    loops = np.arange(n, dtype=np.int64)
    src = np.concatenate([src, loops])
    dst = np.concatenate([dst, loops])

    deg = np.bincount(dst, minlength=n).astype(np.float64)
    dinv = (1.0 / np.sqrt(deg)).astype(np.float32)  # deg >= 1 via self-loops

    core = dst // SR
    dl = dst - core * SR
    t_loc = dl // P
    ld = (dl - t_loc * P).astype(np.int32)
    ps = (src // SR) * SH + (src - (src // SR) * SR)
    w = ps // WS
    iw = (ps - w * WS).astype(np.int32)
    assert iw.max() < 32768

    nkey = NT * NWIN
    key = (core * nkey + t_loc * NWIN + w).astype(np.int64)
    counts = np.bincount(key, minlength=NCORES * nkey).reshape(NCORES, NT, NWIN)
    K = np.ceil(counts.max(axis=0) / P).astype(np.int64)  # [NT, NWIN]

    chunk_base = np.zeros((NT, NWIN), dtype=np.int64)
    flatK = K.reshape(-1)
    chunk_base.reshape(-1)[1:] = np.cumsum(flatK)[:-1]
    nch = int(flatK.sum())

    # super-tile grouping
    st_tiles = [list(range(s, min(s + d.st, NT))) for s in range(0, NT, d.st)]
    n_st = len(st_tiles)
    icols = np.zeros((n_st, NWIN), dtype=np.int64)
    for si, ts in enumerate(st_tiles):
        for wi in range(NWIN):
            icols[si, wi] = int(K[ts, wi].sum()) * (P // 16)
    icol_base = np.zeros((n_st, NWIN), dtype=np.int64)
    icol_base.reshape(-1)[1:] = np.cumsum(icols.reshape(-1))[:-1]
    icols_total = int(icols.sum())

    # seg_off[t, w]: edge offset (within call (st,w)) of tile t's segment
    seg_off = np.zeros((NT, NWIN), dtype=np.int64)
    for si, ts in enumerate(st_tiles):
        for wi in range(NWIN):
            off = 0
            for t in ts:
                seg_off[t, wi] = off
                off += int(K[t, wi]) * P

    st_of_tile = np.zeros(NT, dtype=np.int64)
    for si, ts in enumerate(st_tiles):
        for t in ts:
            st_of_tile[t] = si

    plan = Plan(dims=d, K=K, chunk_base=chunk_base, nch=nch, icols=icols,
                icol_base=icol_base, n_st=n_st, st_tiles=st_tiles)
    plan.dinv_full = dinv

    # rank of each edge within its (core, t, w) group
    order = np.argsort(key, kind="stable")
    sk = key[order]
    newg = np.ones(len(sk), dtype=bool)
    newg[1:] = sk[1:] != sk[:-1]
    starts = np.flatnonzero(newg)
    # rank = position - group start position
    pos = np.arange(len(sk))
    gsp = np.zeros(len(sk), dtype=np.int64)
    gsp[starts] = pos[starts]
    gsp = np.maximum.accumulate(gsp)
    rank_sorted = pos - gsp
    rank = np.empty(len(sk), dtype=np.int64)
    rank[order] = rank_sorted

    batch = np.asarray(batch, dtype=np.int64)

    for c in range(NCORES):
        m = core == c
        t_c, w_c, r_c = t_loc[m], w[m], rank[m]
        iw_c, ld_c = iw[m], ld[m]

        # idx array
        eidx = np.zeros((16, icols_total), dtype=np.int16)
        j = seg_off[t_c, w_c] + r_c  # position within call (st, w)
        col = icol_base[st_of_tile[t_c], w_c] + j // 16
        eidx[j % 16, col] = iw_c
        eidx = np.tile(eidx, (8, 1))

        # ld array (pad = -1)
        eld = np.full((P, nch), -1.0, dtype=np.float32)
        chunkcol = chunk_base[t_c, w_c] + r_c // P
        eld[r_c % P, chunkcol] = ld_c.astype(np.float32)

        plan.eidx.append(eidx)
        plan.eld.append(eld)

        # per-node arrays
        lo, hi = c * SR, (c + 1) * SR
        dv = np.zeros(SH, np.float32)
        dv[:SR] = dinv[lo:hi]
        plan.dinv2d.append(np.ascontiguousarray(
            dv.reshape(NT, P).T).astype(np.float32))
        bq = np.zeros(SH, np.float32)
        bq[:SR] = np.sqrt(deg[lo:hi])
        plan.bq.append(bq.reshape(1, SH).astype(ml_dtypes.bfloat16))
        bv = np.full(SH, -1.0, np.float32)
        bv[:SR] = batch[lo:hi].astype(np.float32)
        plan.batchval.append(np.ascontiguousarray(bv.reshape(NT, P).T))

    cnt = np.bincount(batch, minlength=d.n_graphs).astype(np.float32)
    cnt_inv = 1.0 / np.maximum(cnt, 1.0)
    plan.cnt_inv = np.ascontiguousarray(
        cnt_inv.reshape(d.n_graphs // P, P).T).astype(np.float32)
    return plan


def build_nc(plan: Plan):
    d = plan.dims
    SH, NT, WS, NG = d.shard, d.nt, d.ws, d.n_graphs
    KIN, HID, OUT = d.kin, d.hid, d.out
    NK = KIN // P
    NGT = NG // P
    K, chunk_base = plan.K, plan.chunk_base
    icols_total = int(plan.icols.sum())

    nc = bacc.Bacc("TRN2", target_bir_lowering=False, debug=False)

    x_t = nc.dram_tensor("x_t", [KIN, SH], BF16, kind="ExternalInput")
    eidx = nc.dram_tensor("eidx", [P, icols_total], I16, kind="ExternalInput")
    eld = nc.dram_tensor("eld", [P, plan.nch], F32, kind="ExternalInput")
    dinv_in = nc.dram_tensor("dinv", [P, NT], F32, kind="ExternalInput")
    bq_in = nc.dram_tensor("bq", [1, SH], BF16, kind="ExternalInput")
    bval_in = nc.dram_tensor("bval", [P, NT], F32, kind="ExternalInput")
    w1_in = nc.dram_tensor("w1", [KIN, HID], BF16, kind="ExternalInput")
    w2_in = nc.dram_tensor("w2", [HID, HID], BF16, kind="ExternalInput")
    b1_in = nc.dram_tensor("b1", [1, HID], BF16, kind="ExternalInput")
    b2_in = nc.dram_tensor("b2", [1, HID], BF16, kind="ExternalInput")
    fcw_in = nc.dram_tensor("fcw", [HID, OUT], F32, kind="ExternalInput")
    fcbt_in = nc.dram_tensor("fcbt", [P, OUT], F32, kind="ExternalInput")
    iota128_in = nc.dram_tensor("iota128", [P, P], F32, kind="ExternalInput")
    iotag_in = nc.dram_tensor("iotag", [P, NG], F32, kind="ExternalInput")
    ident_in = nc.dram_tensor("ident", [P, P], BF16, kind="ExternalInput")
    cntinv_in = nc.dram_tensor("cntinv", [P, NGT], F32, kind="ExternalInput")
    out_t = nc.dram_tensor("out", [NG, OUT], F32, kind="ExternalOutput")

    with tile.TileContext(nc) as tc:
        with tc.tile_pool(name="const", bufs=1) as cp, \
             tc.tile_pool(name="xio", bufs=3) as xio, \
             tc.tile_pool(name="stage", bufs=3) as stp, \
             tc.tile_pool(name="gpool", bufs=2) as gp, \
             tc.tile_pool(name="spool", bufs=6) as sp, \
             tc.tile_pool(name="psA", bufs=2, space="PSUM") as psA, \
             tc.tile_pool(name="psB", bufs=2, space="PSUM") as psB, \
             tc.tile_pool(name="psC", bufs=2, space="PSUM") as psC, \
             tc.tile_pool(name="psPool", bufs=1, space="PSUM") as psP, \
             tc.tile_pool(name="dram", bufs=1, space="DRAM") as dp:

            # ---- constants ----
            w1_sb = cp.tile([P, NK, HID], BF16)
            nc.sync.dma_start(out=w1_sb[:], in_=w1_in.rearrange(
                "(k p) h -> p k h", p=P))
            w2_sb = cp.tile([P, HID], BF16)
            nc.sync.dma_start(out=w2_sb[:], in_=w2_in[:])
            b1_sb = cp.tile([1, HID], BF16)
            nc.sync.dma_start(out=b1_sb[:], in_=b1_in[:])
            b2_sb = cp.tile([1, HID], BF16)
            nc.sync.dma_start(out=b2_sb[:], in_=b2_in[:])
            bq_sb = cp.tile([1, SH], BF16)
            nc.sync.dma_start(out=bq_sb[:], in_=bq_in[:])
            dinv_sb = cp.tile([P, NT], F32)
            nc.sync.dma_start(out=dinv_sb[:], in_=dinv_in[:])
            bval_sb = cp.tile([P, NT], F32)
            nc.sync.dma_start(out=bval_sb[:], in_=bval_in[:])
            iota_sb = cp.tile([P, P], F32)
            nc.sync.dma_start(out=iota_sb[:], in_=iota128_in[:])
            iotag_sb = cp.tile([P, NG], F32)
            nc.sync.dma_start(out=iotag_sb[:], in_=iotag_in[:])
            ident_sb = cp.tile([P, P], BF16)
            nc.sync.dma_start(out=ident_sb[:], in_=ident_in[:])
            fcw_sb = cp.tile([P, OUT], F32)
            nc.sync.dma_start(out=fcw_sb[:], in_=fcw_in[:])
            fcbt_sb = cp.tile([P, OUT], F32)
            nc.sync.dma_start(out=fcbt_sb[:], in_=fcbt_in[:])
            cntinv_sb = cp.tile([P, NGT], F32)
            nc.sync.dma_start(out=cntinv_sb[:], in_=cntinv_in[:])
            eld_sb = cp.tile([P, plan.nch], F32)
            nc.sync.dma_start(out=eld_sb[:], in_=eld[:])

            shard1 = dp.tile([SH, HID], BF16)
            shard2 = dp.tile([SH, HID], BF16)
            table1 = dp.tile([d.npad, HID], BF16, addr_space="Shared")
            table2 = dp.tile([d.npad, HID], BF16, addr_space="Shared")
            poolpart = dp.tile([P, NG], F32)
            poolsum = dp.tile([P, NG], F32, addr_space="Shared")

            # ---- stage 1: p1' = dinv * (x @ W1), write shard1 ----
            xr = x_t.rearrange("(k p) n -> p k n", p=P)
            for t in range(NT):
                xt = xio.tile([P, NK, P], BF16, tag="xt")
                nc.sync.dma_start(out=xt[:], in_=xr[:, :, t * P:(t + 1) * P])
                ps = psA.tile([P, HID], F32, space="PSUM", tag="acc")
                for k in range(NK):
                    nc.tensor.matmul(ps[:], xt[:, k, :], w1_sb[:, k, :],
                                     start=(k == 0), stop=(k == NK - 1))
                p1p = xio.tile([P, HID], BF16, tag="p1p")
                nc.scalar.activation(out=p1p[:], in_=ps[:],
                                     func=mybir.ActivationFunctionType.Copy,
                                     scale=dinv_sb[:, t:t + 1])
                nc.sync.dma_start(out=shard1[t * P:(t + 1) * P, :], in_=p1p[:])

            nc.gpsimd.collective_compute(
                "AllGather", mybir.AluOpType.bypass,
                replica_groups=[list(range(NCORES))],
                ins=[shard1.opt()], outs=[table1.opt()])

            # ---- conv aggregation stage (used twice) ----
            def conv_stage(table, b_sb, is_conv2):
                for si, ts in enumerate(plan.st_tiles):
                    gts = []
                    for w in range(NWIN):
                        nchunks = int(K[ts, w].sum())
                        if nchunks == 0:
                            gts.append(None)
                            continue
                        ic = int(plan.icols[si, w])
                        ib = int(plan.icol_base[si, w])
                        idxs = sp.tile([P, ic], I16, tag="idx")
                        nc.sync.dma_start(out=idxs[:], in_=eidx[:, ib:ib + ic])
                        g = gp.tile([P, nchunks, HID], BF16, tag=f"g{w}")
                        # HW cap: <=1024 idxs (64 descs x 16 engines) per call
                        for c0 in range(0, nchunks, 8):
                            c1 = min(nchunks, c0 + 8)
                            nidx = (c1 - c0) * P
                            nc.gpsimd.dma_gather(
                                g[:, c0:c1, :], table[w * WS:(w + 1) * WS, :],
                                idxs[:, c0 * 8:c1 * 8], nidx, nidx, HID)
                        gts.append(g)

                    for t in ts:
                        ps = psA.tile([P, HID], F32, space="PSUM", tag="acc")
                        first = True
                        for w in range(NWIN):
                            kw = int(K[t, w])
                            if kw == 0:
                                continue
                            # chunk offset of tile t within g tile of window w
                            base_in_g = sum(int(K[tt, w]) for tt in ts if tt < t)
                            cb = int(chunk_base[t, w])
                            g = gts[w]
                            for ci in range(kw):
                                s = sp.tile([P, P], BF16, tag="s")
                                col = cb + ci
                                nc.vector.tensor_scalar(
                                    out=s[:], in0=iota_sb[:],
                                    scalar1=eld_sb[:, col:col + 1], scalar2=None,
                                    op0=mybir.AluOpType.is_equal)
                                nc.tensor.matmul(
                                    ps[:], s[:], g[:, base_in_g + ci, :],
                                    start=first, stop=False)
                                first = False
                        # bias as rank-1 outer(sqrt(deg), b): after the
                        # dinv post-scale this contributes exactly +b
                        nc.tensor.matmul(
                            ps[:], bq_sb[0:1, t * P:(t + 1) * P], b_sb[0:1, :],
                            start=False, stop=True)

                        h = stp.tile([P, HID], BF16, tag="h")
                        nc.scalar.activation(
                            out=h[:], in_=ps[:],
                            func=mybir.ActivationFunctionType.Relu,
                            scale=dinv_sb[:, t:t + 1])

                        if not is_conv2:
                            # h1 -> h1T -> p2' = dinv * (h1 @ W2) -> shard2
                            pst = psB.tile([P, HID], BF16, space="PSUM", tag="tmp")
                            nc.tensor.transpose(pst[:], h[:], ident_sb[:])
                            ht = stp.tile([P, HID], BF16, tag="ht")
                            nc.vector.tensor_copy(out=ht[:], in_=pst[:])
                            ps2 = psC.tile([P, HID], F32, space="PSUM", tag="p2")
                            nc.tensor.matmul(ps2[:], ht[:], w2_sb[:],
                                             start=True, stop=True)
                            p2p = stp.tile([P, HID], BF16, tag="p2p")
                            nc.scalar.activation(
                                out=p2p[:], in_=ps2[:],
                                func=mybir.ActivationFunctionType.Copy,
                                scale=dinv_sb[:, t:t + 1])
                            nc.sync.dma_start(
                                out=shard2[t * P:(t + 1) * P, :], in_=p2p[:])
                        else:
                            # mean-pool partial sums: poolT[f, g] += h2^T @ 1hot
                            spool = sp.tile([P, NG], BF16, tag="spool")
                            nc.vector.tensor_scalar(
                                out=spool[:], in0=iotag_sb[:],
                                scalar1=bval_sb[:, t:t + 1], scalar2=None,
                                op0=mybir.AluOpType.is_equal)
                            nc.tensor.matmul(
                                pool_ps[:], h[:], spool[:],
                                start=(t == 0), stop=(t == NT - 1))

            conv_stage(table1, b1_sb, False)

            nc.gpsimd.collective_compute(
                "AllGather", mybir.AluOpType.bypass,
                replica_groups=[list(range(NCORES))],
                ins=[shard2.opt()], outs=[table2.opt()])

            pool_ps = psP.tile([P, NG], F32, space="PSUM")
            conv_stage(table2, b2_sb, True)

            # ---- pool partials -> AllReduce ----
            poolsb = stp.tile([P, NG], F32, tag="poolsb")
            nc.scalar.activation(out=poolsb[:], in_=pool_ps[:],
                                 func=mybir.ActivationFunctionType.Copy)
            nc.sync.dma_start(out=poolpart[:], in_=poolsb[:])
            nc.gpsimd.collective_compute(
                "AllReduce", mybir.AluOpType.add,
                replica_groups=[list(range(NCORES))],
                ins=[poolpart.opt()], outs=[poolsum.opt()])
            sums = cp.tile([P, NG], F32)
            nc.sync.dma_start(out=sums[:], in_=poolsum[:])

            # ---- head: logits + log_softmax ----
            for gt in range(NGT):
                psl = psB.tile([P, OUT], F32, space="PSUM", tag="tmp")
                nc.tensor.matmul(psl[:], sums[:, gt * P:(gt + 1) * P],
                                 fcw_sb[:], start=True, stop=True)
                lg = stp.tile([P, OUT], F32, tag="lg")
                nc.scalar.activation(out=lg[:], in_=psl[:],
                                     func=mybir.ActivationFunctionType.Copy,
                                     scale=cntinv_sb[:, gt:gt + 1])
                lgb = stp.tile([P, OUT], F32, tag="lgb")
                nc.vector.tensor_tensor(out=lgb[:], in0=lg[:], in1=fcbt_sb[:],
                                        op=mybir.AluOpType.add)
                mx = stp.tile([P, 1], F32, tag="mx")
                nc.vector.reduce_max(mx[:], lgb[:], axis=mybir.AxisListType.X)
                zc = stp.tile([P, OUT], F32, tag="zc")
                nc.vector.tensor_scalar(out=zc[:], in0=lgb[:], scalar1=mx[:, 0:1],
                                        scalar2=None,
                                        op0=mybir.AluOpType.subtract)
                ex = stp.tile([P, OUT], F32, tag="ex")
                nc.scalar.activation(out=ex[:], in_=zc[:],
                                     func=mybir.ActivationFunctionType.Exp)
                sm = stp.tile([P, 1], F32, tag="sm")
                nc.vector.reduce_sum(sm[:], ex[:], axis=mybir.AxisListType.X)
                ls = stp.tile([P, 1], F32, tag="ls")
                nc.scalar.activation(out=ls[:], in_=sm[:],
                                     func=mybir.ActivationFunctionType.Ln)
                res = stp.tile([P, OUT], F32, tag="res")
                nc.vector.tensor_scalar(out=res[:], in0=zc[:], scalar1=ls[:, 0:1],
                                        scalar2=None,
                                        op0=mybir.AluOpType.subtract)
                nc.sync.dma_start(out=out_t[gt * P:(gt + 1) * P, :], in_=res[:])

    nc.compile()
    return nc


def make_in_maps(plan: Plan, x, W1, b1, W2, b2, fc_w, fc_b):
    d = plan.dims
    SR, SH, NG = d.shard_real, d.shard, d.n_graphs
    bf = ml_dtypes.bfloat16

    x = np.asarray(x, np.float32)
    w1 = np.asarray(W1, np.float32).astype(bf)
    w2 = np.asarray(W2, np.float32).astype(bf)
    b1a = np.asarray(b1, np.float32).astype(bf).reshape(1, -1)
    b2a = np.asarray(b2, np.float32).astype(bf).reshape(1, -1)
    fcw = np.ascontiguousarray(np.asarray(fc_w, np.float32))
    fcbt = np.tile(np.asarray(fc_b, np.float32).reshape(1, -1), (P, 1))
    iota128 = np.tile(np.arange(P, dtype=np.float32)[None, :], (P, 1))
    iotag = np.tile(np.arange(NG, dtype=np.float32)[None, :], (P, 1))
    ident = np.eye(P, dtype=np.float32).astype(bf)

    in_maps = []
    for c in range(NCORES):
        xs = np.zeros((SH, d.kin), np.float32)
        xs[:SR] = x[c * SR:(c + 1) * SR]
        in_maps.append({
            "x_t": np.ascontiguousarray(xs.T).astype(bf),
            "eidx": plan.eidx[c],
            "eld": plan.eld[c],
            "dinv": plan.dinv2d[c],
            "bq": plan.bq[c],
            "bval": plan.batchval[c],
            "w1": w1, "w2": w2, "b1": b1a, "b2": b2a,
            "fcw": fcw, "fcbt": fcbt,
            "iota128": iota128, "iotag": iotag, "ident": ident,
            "cntinv": plan.cnt_inv,
        })
    return in_maps


def prepare(x, edge_index, batch, W1, b1, W2, b2, fc_w, fc_b, dims=None):
    d = dims or Dims(n_nodes=x.shape[0], n_graphs=512, kin=x.shape[1],
                     hid=W1.shape[1], out=fc_w.shape[1])
    plan = make_plan(np.asarray(edge_index), np.asarray(batch), d)
    nc = build_nc(plan)
    in_maps = make_in_maps(plan, x, W1, b1, W2, b2, fc_w, fc_b)
    return nc, in_maps, plan


def kernel(x, edge_index, batch, W1, b1, W2, b2, fc_w, fc_b):
    nc, in_maps, _ = prepare(np.asarray(x), edge_index, batch,
                             W1, b1, W2, b2, fc_w, fc_b)
    res = run_bass_kernel_spmd(nc, in_maps, list(range(NCORES)))
    return np.ascontiguousarray(res.results[0]["out"].astype(np.float32))


# revision 5
# speedup vs baseline: 1.0134x; 1.0134x over previous
"""GCN classifier (2x GCNConv + mean-pool + linear head) on 8 Trainium2
NeuronCores via Bass/Tile.

Sharding: nodes (and their incident edges, grouped by destination) are
partitioned across the 8 cores; the small weight matrices are replicated;
per-layer node features are AllGathered into a full bf16 table that each
core gathers source rows from; the per-graph mean-pool partial sums are
AllReduced.

GCN normalization is factored as out = dinv * (A @ (dinv * (x @ W))) with
dinv = 1/sqrt(deg+selfloop), so the edge aggregation itself needs no
per-edge scaling: each 128-destination-node tile accumulates
S_chunk^T @ G_chunk in PSUM, where G_chunk is 128 dma_gather'ed source
rows (bf16) and S_chunk is a one-hot selection matrix generated on the
vector engine via iota == dst_local.
"""

import math
from dataclasses import dataclass, field

import ml_dtypes
import numpy as np

import concourse.bacc as bacc
import concourse.mybir as mybir
import concourse.tile as tile
from concourse.bass_utils import run_bass_kernel_spmd

P = 128
NCORES = 8
NWIN = 4  # int16 gather index windows over the node table

BF16 = mybir.dt.bfloat16
F32 = mybir.dt.float32
I16 = mybir.dt.int16


@dataclass
class Dims:
    n_nodes: int = 100000
    n_graphs: int = 512
    kin: int = 384
    hid: int = 128
    out: int = 2
    st: int = 7  # dst tiles per gather super-tile

    @property
    def shard_real(self):
        return self.n_nodes // NCORES

    @property
    def shard(self):  # padded per-core shard (multiple of 128)
        return ((self.shard_real + P - 1) // P) * P

    @property
    def nt(self):  # dst tiles per core
        return self.shard // P

    @property
    def npad(self):
        return self.shard * NCORES

    @property
    def ws(self):  # gather window rows
        assert self.npad % NWIN == 0
        return self.npad // NWIN


@dataclass
class Plan:
    dims: Dims
    K: np.ndarray          # [nt, NWIN] chunks per (tile, window), maxed over cores
    chunk_base: np.ndarray  # [nt, NWIN] first global chunk id of (t, w)
    nch: int               # total chunks per conv
    icols: np.ndarray      # [n_st, NWIN] idx cols per gather call
    icol_base: np.ndarray  # [n_st, NWIN] first idx col of call
    n_st: int
    st_tiles: list         # list of tile-index lists per super-tile
    # per-core input arrays
    eidx: list = field(default_factory=list)   # int16 [128, ICOLS]
    eld: list = field(default_factory=list)    # f32 [128, nch]
    dinv2d: list = field(default_factory=list)  # f32 [128, nt]
    bq: list = field(default_factory=list)     # bf16 [1, shard]
    batchval: list = field(default_factory=list)  # f32 [128, nt]
    cnt_inv: np.ndarray = None  # f32 [128, ng//128]
    dinv_full: np.ndarray = None  # f32 [n_nodes]


def _wrap16(idx_flat, ncols):
    """Flat int16 index list -> [128, ncols] wrapped layout (16-partition
    wrap, replicated over the 8 q7 cores)."""
    arr = np.zeros((16, ncols), dtype=np.int16)
    n = len(idx_flat)
    arr.reshape(-1, order="F").flat[:n] = idx_flat  # i -> [i%16, i//16]
    return np.tile(arr, (8, 1))


def make_plan(edge_index, batch, dims: Dims) -> Plan:
    d = dims
    SR, SH, NT, WS = d.shard_real, d.shard, d.nt, d.ws
    n = d.n_nodes

    src = np.asarray(edge_index[0], dtype=np.int64)
    dst = np.asarray(edge_index[1], dtype=np.int64)
    loops = np.arange(n, dtype=np.int64)
    src = np.concatenate([src, loops])
    dst = np.concatenate([dst, loops])

    deg = np.bincount(dst, minlength=n).astype(np.float64)
    dinv = (1.0 / np.sqrt(deg)).astype(np.float32)  # deg >= 1 via self-loops

    core = dst // SR
    dl = dst - core * SR
    t_loc = dl // P
    ld = (dl - t_loc * P).astype(np.int32)
    ps = (src // SR) * SH + (src - (src // SR) * SR)
    w = ps // WS
    iw = (ps - w * WS).astype(np.int32)
    assert iw.max() < 32768

    nkey = NT * NWIN
    key = (core * nkey + t_loc * NWIN + w).astype(np.int64)
    counts = np.bincount(key, minlength=NCORES * nkey).reshape(NCORES, NT, NWIN)
    K = np.ceil(counts.max(axis=0) / P).astype(np.int64)  # [NT, NWIN]

    chunk_base = np.zeros((NT, NWIN), dtype=np.int64)
    flatK = K.reshape(-1)
    chunk_base.reshape(-1)[1:] = np.cumsum(flatK)[:-1]
    nch = int(flatK.sum())

    # super-tile grouping
    st_tiles = [list(range(s, min(s + d.st, NT))) for s in range(0, NT, d.st)]
    n_st = len(st_tiles)
    icols = np.zeros((n_st, NWIN), dtype=np.int64)
    for si, ts in enumerate(st_tiles):
        for wi in range(NWIN):
            icols[si, wi] = int(K[ts, wi].sum()) * (P // 16)
    icol_base = np.zeros((n_st, NWIN), dtype=np.int64)
    icol_base.reshape(-1)[1:] = np.cumsum(icols.reshape(-1))[:-1]
    icols_total = int(icols.sum())

    # seg_off[t, w]: edge offset (within call (st,w)) of tile t's segment
    seg_off = np.zeros((NT, NWIN), dtype=np.int64)
    for si, ts in enumerate(st_tiles):
        for wi in range(NWIN):
            off = 0
            for t in ts:
                seg_off[t, wi] = off
                off += int(K[t, wi]) * P

    st_of_tile = np.zeros(NT, dtype=np.int64)
    for si, ts in enumerate(st_tiles):
        for t in ts:
            st_of_tile[t] = si

    plan = Plan(dims=d, K=K, chunk_base=chunk_base, nch=nch, icols=icols,
                icol_base=icol_base, n_st=n_st, st_tiles=st_tiles)
    plan.dinv_full = dinv

    # rank of each edge within its (core, t, w) group
    order = np.argsort(key, kind="stable")
    sk = key[order]
    newg = np.ones(len(sk), dtype=bool)
    newg[1:] = sk[1:] != sk[:-1]
    starts = np.flatnonzero(newg)
    # rank = position - group start position
    pos = np.arange(len(sk))
    gsp = np.zeros(len(sk), dtype=np.int64)
    gsp[starts] = pos[starts]
    gsp = np.maximum.accumulate(gsp)
    rank_sorted = pos - gsp
    rank = np.empty(len(sk), dtype=np.int64)
    rank[order] = rank_sorted

    batch = np.asarray(batch, dtype=np.int64)

    for c in range(NCORES):
        m = core == c
        t_c, w_c, r_c = t_loc[m], w[m], rank[m]
        iw_c, ld_c = iw[m], ld[m]

        # idx array
        eidx = np.zeros((16, icols_total), dtype=np.int16)
        j = seg_off[t_c, w_c] + r_c  # position within call (st, w)
        col = icol_base[st_of_tile[t_c], w_c] + j // 16
        eidx[j % 16, col] = iw_c
        eidx = np.tile(eidx, (8, 1))

        # ld array (pad = -1)
        eld = np.full((P, nch), -1.0, dtype=np.float32)
        chunkcol = chunk_base[t_c, w_c] + r_c // P
        eld[r_c % P, chunkcol] = ld_c.astype(np.float32)

        plan.eidx.append(eidx)
        plan.eld.append(eld)

        # per-node arrays
        lo, hi = c * SR, (c + 1) * SR
        dv = np.zeros(SH, np.float32)
        dv[:SR] = dinv[lo:hi]
        plan.dinv2d.append(np.ascontiguousarray(
            dv.reshape(NT, P).T).astype(np.float32))
        bq = np.zeros(SH, np.float32)
        bq[:SR] = np.sqrt(deg[lo:hi])
        plan.bq.append(bq.reshape(1, SH).astype(ml_dtypes.bfloat16))
        bv = np.full(SH, -1.0, np.float32)
        bv[:SR] = batch[lo:hi].astype(np.float32)
        plan.batchval.append(np.ascontiguousarray(bv.reshape(NT, P).T))

    cnt = np.bincount(batch, minlength=d.n_graphs).astype(np.float32)
    cnt_inv = 1.0 / np.maximum(cnt, 1.0)
    plan.cnt_inv = np.ascontiguousarray(
        cnt_inv.reshape(d.n_graphs // P, P).T).astype(np.float32)
    return plan


def build_nc(plan: Plan):
    d = plan.dims
    SH, NT, WS, NG = d.shard, d.nt, d.ws, d.n_graphs
    KIN, HID, OUT = d.kin, d.hid, d.out
    NK = KIN // P
    NGT = NG // P
    K, chunk_base = plan.K, plan.chunk_base
    icols_total = int(plan.icols.sum())

    nc = bacc.Bacc("TRN2", target_bir_lowering=False, debug=False)

    x_t = nc.dram_tensor("x_t", [KIN, SH], BF16, kind="ExternalInput")
    eidx = nc.dram_tensor("eidx", [P, icols_total], I16, kind="ExternalInput")
    eld = nc.dram_tensor("eld", [P, plan.nch], F32, kind="ExternalInput")
    dinv_in = nc.dram_tensor("dinv", [P, NT], F32, kind="ExternalInput")
    bq_in = nc.dram_tensor("bq", [1, SH], BF16, kind="ExternalInput")
    bval_in = nc.dram_tensor("bval", [P, NT], F32, kind="ExternalInput")
    w1_in = nc.dram_tensor("w1", [KIN, HID], BF16, kind="ExternalInput")
    w2_in = nc.dram_tensor("w2", [HID, HID], BF16, kind="ExternalInput")
    b1_in = nc.dram_tensor("b1", [1, HID], BF16, kind="ExternalInput")
    b2_in = nc.dram_tensor("b2", [1, HID], BF16, kind="ExternalInput")
    fcw_in = nc.dram_tensor("fcw", [HID, OUT], F32, kind="ExternalInput")
    fcbt_in = nc.dram_tensor("fcbt", [P, OUT], F32, kind="ExternalInput")
    iota128_in = nc.dram_tensor("iota128", [P, P], BF16, kind="ExternalInput")
    iotag_in = nc.dram_tensor("iotag", [P, NG], F32, kind="ExternalInput")
    ident_in = nc.dram_tensor("ident", [P, P], BF16, kind="ExternalInput")
    cntinv_in = nc.dram_tensor("cntinv", [P, NGT], F32, kind="ExternalInput")
    out_t = nc.dram_tensor("out", [NG, OUT], F32, kind="ExternalOutput")

    with tile.TileContext(nc) as tc:
        with tc.tile_pool(name="const", bufs=1) as cp, \
             tc.tile_pool(name="xio", bufs=3) as xio, \
             tc.tile_pool(name="stage", bufs=3) as stp, \
             tc.tile_pool(name="gpool", bufs=3) as gp, \
             tc.tile_pool(name="spool", bufs=12) as sp, \
             tc.tile_pool(name="psA", bufs=3, space="PSUM") as psA, \
             tc.tile_pool(name="psB", bufs=2, space="PSUM") as psB, \
             tc.tile_pool(name="psC", bufs=2, space="PSUM") as psC, \
             tc.tile_pool(name="psPool", bufs=1, space="PSUM") as psP, \
             tc.tile_pool(name="dram", bufs=1, space="DRAM") as dp:

            # ---- constants ----
            w1_sb = cp.tile([P, NK, HID], BF16)
            nc.sync.dma_start(out=w1_sb[:], in_=w1_in.rearrange(
                "(k p) h -> p k h", p=P))
            w2_sb = cp.tile([P, HID], BF16)
            nc.sync.dma_start(out=w2_sb[:], in_=w2_in[:])
            b1_sb = cp.tile([1, HID], BF16)
            nc.sync.dma_start(out=b1_sb[:], in_=b1_in[:])
            b2_sb = cp.tile([1, HID], BF16)
            nc.sync.dma_start(out=b2_sb[:], in_=b2_in[:])
            bq_sb = cp.tile([1, SH], BF16)
            nc.sync.dma_start(out=bq_sb[:], in_=bq_in[:])
            dinv_sb = cp.tile([P, NT], F32)
            nc.sync.dma_start(out=dinv_sb[:], in_=dinv_in[:])
            bval_sb = cp.tile([P, NT], F32)
            nc.sync.dma_start(out=bval_sb[:], in_=bval_in[:])
            iota_sb = cp.tile([P, P], BF16)
            nc.sync.dma_start(out=iota_sb[:], in_=iota128_in[:])
            iotag_sb = cp.tile([P, NG], F32)
            nc.sync.dma_start(out=iotag_sb[:], in_=iotag_in[:])
            ident_sb = cp.tile([P, P], BF16)
            nc.sync.dma_start(out=ident_sb[:], in_=ident_in[:])
            fcw_sb = cp.tile([P, OUT], F32)
            nc.sync.dma_start(out=fcw_sb[:], in_=fcw_in[:])
            fcbt_sb = cp.tile([P, OUT], F32)
            nc.sync.dma_start(out=fcbt_sb[:], in_=fcbt_in[:])
            cntinv_sb = cp.tile([P, NGT], F32)
            nc.sync.dma_start(out=cntinv_sb[:], in_=cntinv_in[:])
            eld_sb = cp.tile([P, plan.nch], F32)
            nc.sync.dma_start(out=eld_sb[:], in_=eld[:])

            shard1 = dp.tile([SH, HID], BF16)
            shard2 = dp.tile([SH, HID], BF16)
            table1 = dp.tile([d.npad, HID], BF16, addr_space="Shared")
            table2 = dp.tile([d.npad, HID], BF16, addr_space="Shared")
            poolpart = dp.tile([P, NG], F32)
            poolsum = dp.tile([P, NG], F32, addr_space="Shared")

            # ---- stage 1: p1' = dinv * (x @ W1), write shard1 ----
            xr = x_t.rearrange("(k p) n -> p k n", p=P)
            for t in range(NT):
                xt = xio.tile([P, NK, P], BF16, tag="xt")
                nc.sync.dma_start(out=xt[:], in_=xr[:, :, t * P:(t + 1) * P])
                ps = psA.tile([P, HID], F32, space="PSUM", tag="acc")
                for k in range(NK):
                    nc.tensor.matmul(ps[:], xt[:, k, :], w1_sb[:, k, :],
                                     start=(k == 0), stop=(k == NK - 1))
                p1p = xio.tile([P, HID], BF16, tag="p1p")
                nc.scalar.activation(out=p1p[:], in_=ps[:],
                                     func=mybir.ActivationFunctionType.Copy,
                                     scale=dinv_sb[:, t:t + 1])
                nc.sync.dma_start(out=shard1[t * P:(t + 1) * P, :], in_=p1p[:])

            nc.gpsimd.collective_compute(
                "AllGather", mybir.AluOpType.bypass,
                replica_groups=[list(range(NCORES))],
                ins=[shard1.opt()], outs=[table1.opt()])

            # ---- conv aggregation stage (used twice) ----
            def conv_stage(table, b_sb, is_conv2):
                for si, ts in enumerate(plan.st_tiles):
                    gts = []
                    for w in range(NWIN):
                        nchunks = int(K[ts, w].sum())
                        if nchunks == 0:
                            gts.append(None)
                            continue
                        ic = int(plan.icols[si, w])
                        ib = int(plan.icol_base[si, w])
                        idxs = sp.tile([P, ic], I16, tag="idx")
                        nc.sync.dma_start(out=idxs[:], in_=eidx[:, ib:ib + ic])
                        g = gp.tile([P, nchunks, HID], BF16, tag=f"g{w}")
                        # HW cap: <=1024 idxs (64 descs x 16 engines) per call
                        for c0 in range(0, nchunks, 8):
                            c1 = min(nchunks, c0 + 8)
                            nidx = (c1 - c0) * P
                            nc.gpsimd.dma_gather(
                                g[:, c0:c1, :], table[w * WS:(w + 1) * WS, :],
                                idxs[:, c0 * 8:c1 * 8], nidx, nidx, HID)
                        gts.append(g)

                    for t in ts:
                        ps = psA.tile([P, HID], F32, space="PSUM", tag="acc")
                        first = True
                        for w in range(NWIN):
                            kw = int(K[t, w])
                            if kw == 0:
                                continue
                            # chunk offset of tile t within g tile of window w
                            base_in_g = sum(int(K[tt, w]) for tt in ts if tt < t)
                            cb = int(chunk_base[t, w])
                            g = gts[w]
                            for ci in range(kw):
                                s = sp.tile([P, P], BF16, tag="s")
                                col = cb + ci
                                nc.vector.tensor_scalar(
                                    out=s[:], in0=iota_sb[:],
                                    scalar1=eld_sb[:, col:col + 1], scalar2=None,
                                    op0=mybir.AluOpType.is_equal)
                                nc.tensor.matmul(
                                    ps[:], s[:], g[:, base_in_g + ci, :],
                                    start=first, stop=False)
                                first = False
                        # bias as rank-1 outer(sqrt(deg), b): after the
                        # dinv post-scale this contributes exactly +b
                        nc.tensor.matmul(
                            ps[:], bq_sb[0:1, t * P:(t + 1) * P], b_sb[0:1, :],
                            start=False, stop=True)

                        h = stp.tile([P, HID], BF16, tag="h")
                        nc.scalar.activation(
                            out=h[:], in_=ps[:],
                            func=mybir.ActivationFunctionType.Relu,
                            scale=dinv_sb[:, t:t + 1])

                        if not is_conv2:
                            # h1 -> h1T -> p2' = dinv * (h1 @ W2) -> shard2
                            pst = psB.tile([P, HID], BF16, space="PSUM", tag="tmp")
                            nc.tensor.transpose(pst[:], h[:], ident_sb[:])
                            ht = stp.tile([P, HID], BF16, tag="ht")
                            nc.vector.tensor_copy(out=ht[:], in_=pst[:])
                            ps2 = psC.tile([P, HID], F32, space="PSUM", tag="p2")
                            nc.tensor.matmul(ps2[:], ht[:], w2_sb[:],
                                             start=True, stop=True)
                            p2p = stp.tile([P, HID], BF16, tag="p2p")
                            nc.scalar.activation(
                                out=p2p[:], in_=ps2[:],
                                func=mybir.ActivationFunctionType.Copy,
                                scale=dinv_sb[:, t:t + 1])
                            nc.sync.dma_start(
                                out=shard2[t * P:(t + 1) * P, :], in_=p2p[:])
                        else:
                            # mean-pool partial sums: poolT[f, g] += h2^T @ 1hot
                            spool = sp.tile([P, NG], BF16, tag="spool")
                            nc.vector.tensor_scalar(
                                out=spool[:], in0=iotag_sb[:],
                                scalar1=bval_sb[:, t:t + 1], scalar2=None,
                                op0=mybir.AluOpType.is_equal)
                            nc.tensor.matmul(
                                pool_ps[:], h[:], spool[:],
                                start=(t == 0), stop=(t == NT - 1))

            conv_stage(table1, b1_sb, False)

            nc.gpsimd.collective_compute(
                "AllGather", mybir.AluOpType.bypass,
                replica_groups=[list(range(NCORES))],
                ins=[shard2.opt()], outs=[table2.opt()])

            pool_ps = psP.tile([P, NG], F32, space="PSUM")
            conv_stage(table2, b2_sb, True)

            # ---- pool partials -> AllReduce ----
            poolsb = stp.tile([P, NG], F32, tag="poolsb")
            nc.scalar.activation(out=poolsb[:], in_=pool_ps[:],
                                 func=mybir.ActivationFunctionType.Copy)
            nc.sync.dma_start(out=poolpart[:], in_=poolsb[:])
            nc.gpsimd.collective_compute(
                "AllReduce", mybir.AluOpType.add,
                replica_groups=[list(range(NCORES))],
                ins=[poolpart.opt()], outs=[poolsum.opt()])
            sums = cp.tile([P, NG], F32)
            nc.sync.dma_start(out=sums[:], in_=poolsum[:])

            # ---- head: logits + log_softmax ----
            for gt in range(NGT):
                psl = psB.tile([P, OUT], F32, space="PSUM", tag="tmp")
                nc.tensor.matmul(psl[:], sums[:, gt * P:(gt + 1) * P],
                                 fcw_sb[:], start=True, stop=True)
                lg = stp.tile([P, OUT], F32, tag="lg")
                nc.scalar.activation(out=lg[:], in_=psl[:],
                                     func=mybir.ActivationFunctionType.Copy,
                                     scale=cntinv_sb[:, gt:gt + 1])
                lgb = stp.tile([P, OUT], F32, tag="lgb")
                nc.vector.tensor_tensor(out=lgb[:], in0=lg[:], in1=fcbt_sb[:],
                                        op=mybir.AluOpType.add)
                mx = stp.tile([P, 1], F32, tag="mx")
                nc.vector.reduce_max(mx[:], lgb[:], axis=mybir.AxisListType.X)
                zc = stp.tile([P, OUT], F32, tag="zc")
                nc.vector.tensor_scalar(out=zc[:], in0=lgb[:], scalar1=mx[:, 0:1],
                                        scalar2=None,
                                        op0=mybir.AluOpType.subtract)
                ex = stp.tile([P, OUT], F32, tag="ex")
                nc.scalar.activation(out=ex[:], in_=zc[:],
                                     func=mybir.ActivationFunctionType.Exp)
                sm = stp.tile([P, 1], F32, tag="sm")
                nc.vector.reduce_sum(sm[:], ex[:], axis=mybir.AxisListType.X)
                ls = stp.tile([P, 1], F32, tag="ls")
                nc.scalar.activation(out=ls[:], in_=sm[:],
                                     func=mybir.ActivationFunctionType.Ln)
                res = stp.tile([P, OUT], F32, tag="res")
                nc.vector.tensor_scalar(out=res[:], in0=zc[:], scalar1=ls[:, 0:1],
                                        scalar2=None,
                                        op0=mybir.AluOpType.subtract)
                nc.sync.dma_start(out=out_t[gt * P:(gt + 1) * P, :], in_=res[:])

    nc.compile()
    return nc


def make_in_maps(plan: Plan, x, W1, b1, W2, b2, fc_w, fc_b):
    d = plan.dims
    SR, SH, NG = d.shard_real, d.shard, d.n_graphs
    bf = ml_dtypes.bfloat16

    x = np.asarray(x, np.float32)
    w1 = np.asarray(W1, np.float32).astype(bf)
    w2 = np.asarray(W2, np.float32).astype(bf)
    b1a = np.asarray(b1, np.float32).astype(bf).reshape(1, -1)
    b2a = np.asarray(b2, np.float32).astype(bf).reshape(1, -1)
    fcw = np.ascontiguousarray(np.asarray(fc_w, np.float32))
    fcbt = np.tile(np.asarray(fc_b, np.float32).reshape(1, -1), (P, 1))
    iota128 = np.tile(np.arange(P, dtype=np.float32)[None, :], (P, 1)).astype(bf)
    iotag = np.tile(np.arange(NG, dtype=np.float32)[None, :], (P, 1))
    ident = np.eye(P, dtype=np.float32).astype(bf)

    in_maps = []
    for c in range(NCORES):
        xs = np.zeros((SH, d.kin), np.float32)
        xs[:SR] = x[c * SR:(c + 1) * SR]
        in_maps.append({
            "x_t": np.ascontiguousarray(xs.T).astype(bf),
            "eidx": plan.eidx[c],
            "eld": plan.eld[c],
            "dinv": plan.dinv2d[c],
            "bq": plan.bq[c],
            "bval": plan.batchval[c],
            "w1": w1, "w2": w2, "b1": b1a, "b2": b2a,
            "fcw": fcw, "fcbt": fcbt,
            "iota128": iota128, "iotag": iotag, "ident": ident,
            "cntinv": plan.cnt_inv,
        })
    return in_maps


def prepare(x, edge_index, batch, W1, b1, W2, b2, fc_w, fc_b, dims=None):
    d = dims or Dims(n_nodes=x.shape[0], n_graphs=512, kin=x.shape[1],
                     hid=W1.shape[1], out=fc_w.shape[1])
    plan = make_plan(np.asarray(edge_index), np.asarray(batch), d)
    nc = build_nc(plan)
    in_maps = make_in_maps(plan, x, W1, b1, W2, b2, fc_w, fc_b)
    return nc, in_maps, plan


def kernel(x, edge_index, batch, W1, b1, W2, b2, fc_w, fc_b):
    nc, in_maps, _ = prepare(np.asarray(x), edge_index, batch,
                             W1, b1, W2, b2, fc_w, fc_b)
    res = run_bass_kernel_spmd(nc, in_maps, list(range(NCORES)))
    return np.ascontiguousarray(res.results[0]["out"].astype(np.float32))


# revision 9
# speedup vs baseline: 1.1429x; 1.1278x over previous
"""GCN classifier (2x GCNConv + mean-pool + linear head) on 8 Trainium2
NeuronCores via Bass/Tile.

Sharding: nodes (and their incident edges, grouped by destination) are
partitioned across the 8 cores; the small weight matrices are replicated;
per-layer node features are AllGathered into a full bf16 table that each
core gathers source rows from; the per-graph mean-pool partial sums are
AllReduced.

GCN normalization is factored as out = dinv * (A @ (dinv * (x @ W))) with
dinv = 1/sqrt(deg+selfloop), so the edge aggregation itself needs no
per-edge scaling: each 128-destination-node tile accumulates
S_chunk^T @ G_chunk in PSUM, where G_chunk is 128 dma_gather'ed source
rows (bf16) and S_chunk is a one-hot selection matrix generated on the
vector engine via iota == dst_local.
"""

import math
from dataclasses import dataclass, field

import ml_dtypes
import numpy as np

import concourse.bacc as bacc
import concourse.mybir as mybir
import concourse.tile as tile
from concourse.bass_utils import run_bass_kernel_spmd

P = 128
NCORES = 8
NWIN = 4  # int16 gather index windows over the node table

BF16 = mybir.dt.bfloat16
F32 = mybir.dt.float32
I16 = mybir.dt.int16


@dataclass
class Dims:
    n_nodes: int = 100000
    n_graphs: int = 512
    kin: int = 384
    hid: int = 128
    out: int = 2
    st: int = 7  # dst tiles per gather super-tile

    @property
    def shard_real(self):
        return self.n_nodes // NCORES

    @property
    def shard(self):  # padded per-core shard (multiple of 128)
        return ((self.shard_real + P - 1) // P) * P

    @property
    def nt(self):  # dst tiles per core
        return self.shard // P

    @property
    def npad(self):
        return self.shard * NCORES

    @property
    def ws(self):  # gather window rows
        assert self.npad % NWIN == 0
        return self.npad // NWIN


@dataclass
class Plan:
    dims: Dims
    K: np.ndarray          # [nt, NWIN] chunks per (tile, window), maxed over cores
    chunk_base: np.ndarray  # [nt, NWIN] first global chunk id of (t, w)
    nch: int               # total chunks per conv
    icols: np.ndarray      # [n_st, NWIN] idx cols per gather call
    icol_base: np.ndarray  # [n_st, NWIN] first idx col of call
    n_st: int
    st_tiles: list         # list of tile-index lists per super-tile
    # per-core input arrays
    eidx: list = field(default_factory=list)   # int16 [128, ICOLS]
    eld: list = field(default_factory=list)    # f32 [128, nch]
    dinv2d: list = field(default_factory=list)  # f32 [128, nt]
    bq: list = field(default_factory=list)     # bf16 [1, shard]
    batchval: list = field(default_factory=list)  # f32 [128, nt]
    cnt_inv: np.ndarray = None  # f32 [128, ng//128]
    dinv_full: np.ndarray = None  # f32 [n_nodes]
    maxcnt: np.ndarray = None    # [nt, NWIN] real max edges per (t, w)
    tw_col_off: np.ndarray = None  # [nt, NWIN] absolute idx col offsets


def _wrap16(idx_flat, ncols):
    """Flat int16 index list -> [128, ncols] wrapped layout (16-partition
    wrap, replicated over the 8 q7 cores)."""
    arr = np.zeros((16, ncols), dtype=np.int16)
    n = len(idx_flat)
    arr.reshape(-1, order="F").flat[:n] = idx_flat  # i -> [i%16, i//16]
    return np.tile(arr, (8, 1))


def make_plan(edge_index, batch, dims: Dims) -> Plan:
    d = dims
    SR, SH, NT, WS = d.shard_real, d.shard, d.nt, d.ws
    n = d.n_nodes

    src = np.asarray(edge_index[0], dtype=np.int64)
    dst = np.asarray(edge_index[1], dtype=np.int64)
    loops = np.arange(n, dtype=np.int64)
    # degree includes the self-loop; the self contribution itself is added
    # on-chip from the local pre-scaled rows, so loops stay OUT of the
    # gathered edge lists.
    deg = (np.bincount(dst, minlength=n) + 1).astype(np.float64)
    dinv = (1.0 / np.sqrt(deg)).astype(np.float32)

    core = dst // SR
    dl = dst - core * SR
    t_loc = dl // P
    ld = (dl - t_loc * P).astype(np.int32)
    ps = (src // SR) * SH + (src - (src // SR) * SR)
    w = ps // WS
    iw = (ps - w * WS).astype(np.int32)
    assert iw.max() < 32768

    nkey = NT * NWIN
    key = (core * nkey + t_loc * NWIN + w).astype(np.int64)
    counts = np.bincount(key, minlength=NCORES * nkey).reshape(NCORES, NT, NWIN)
    maxcnt = counts.max(axis=0).astype(np.int64)          # [NT, NWIN] real max
    K = np.ceil(maxcnt / P).astype(np.int64)              # [NT, NWIN] chunks

    chunk_base = np.zeros((NT, NWIN), dtype=np.int64)
    flatK = K.reshape(-1)
    chunk_base.reshape(-1)[1:] = np.cumsum(flatK)[:-1]
    nch = int(flatK.sum())

    # super-tile grouping
    st_tiles = [list(range(s, min(s + d.st, NT))) for s in range(0, NT, d.st)]
    n_st = len(st_tiles)
    tw_cols = np.ceil(maxcnt / 16).astype(np.int64)       # idx cols per (t,w)
    icols = np.zeros((n_st, NWIN), dtype=np.int64)
    for si, ts in enumerate(st_tiles):
        for wi in range(NWIN):
            icols[si, wi] = int(tw_cols[ts, wi].sum())
    icol_base = np.zeros((n_st, NWIN), dtype=np.int64)
    icol_base.reshape(-1)[1:] = np.cumsum(icols.reshape(-1))[:-1]
    icols_total = int(icols.sum())

    # tw_col_off[t, w]: absolute idx col where (t,w)'s segment starts
    tw_col_off = np.zeros((NT, NWIN), dtype=np.int64)
    for si, ts in enumerate(st_tiles):
        for wi in range(NWIN):
            off = int(icol_base[si, wi])
            for t in ts:
                tw_col_off[t, wi] = off
                off += int(tw_cols[t, wi])

    st_of_tile = np.zeros(NT, dtype=np.int64)
    for si, ts in enumerate(st_tiles):
        for t in ts:
            st_of_tile[t] = si

    plan = Plan(dims=d, K=K, chunk_base=chunk_base, nch=nch, icols=icols,
                icol_base=icol_base, n_st=n_st, st_tiles=st_tiles)
    plan.maxcnt = maxcnt
    plan.tw_col_off = tw_col_off
    plan.dinv_full = dinv

    # rank of each edge within its (core, t, w) group
    order = np.argsort(key, kind="stable")
    sk = key[order]
    newg = np.ones(len(sk), dtype=bool)
    newg[1:] = sk[1:] != sk[:-1]
    starts = np.flatnonzero(newg)
    # rank = position - group start position
    pos = np.arange(len(sk))
    gsp = np.zeros(len(sk), dtype=np.int64)
    gsp[starts] = pos[starts]
    gsp = np.maximum.accumulate(gsp)
    rank_sorted = pos - gsp
    rank = np.empty(len(sk), dtype=np.int64)
    rank[order] = rank_sorted

    batch = np.asarray(batch, dtype=np.int64)

    for c in range(NCORES):
        m = core == c
        t_c, w_c, r_c = t_loc[m], w[m], rank[m]
        iw_c, ld_c = iw[m], ld[m]

        # idx array: one gather call per (t, w); idx i of the call sits at
        # [i % 16, call_col_off + i // 16]
        eidx = np.zeros((16, icols_total), dtype=np.int16)
        col = tw_col_off[t_c, w_c] + r_c // 16
        eidx[r_c % 16, col] = iw_c
        eidx = np.tile(eidx, (8, 1))

        # ld array (pad = -1)
        eld = np.full((P, nch), -1.0, dtype=np.float32)
        chunkcol = chunk_base[t_c, w_c] + r_c // P
        eld[r_c % P, chunkcol] = ld_c.astype(np.float32)

        plan.eidx.append(eidx)
        plan.eld.append(eld)

        # per-node arrays
        lo, hi = c * SR, (c + 1) * SR
        dv = np.zeros(SH, np.float32)
        dv[:SR] = dinv[lo:hi]
        plan.dinv2d.append(np.ascontiguousarray(
            dv.reshape(NT, P).T).astype(np.float32))
        bq = np.zeros(SH, np.float32)
        bq[:SR] = np.sqrt(deg[lo:hi])
        plan.bq.append(bq.reshape(1, SH).astype(ml_dtypes.bfloat16))
        bv = np.full(SH, -1.0, np.float32)
        bv[:SR] = batch[lo:hi].astype(np.float32)
        plan.batchval.append(np.ascontiguousarray(bv.reshape(NT, P).T))

    cnt = np.bincount(batch, minlength=d.n_graphs).astype(np.float32)
    cnt_inv = 1.0 / np.maximum(cnt, 1.0)
    plan.cnt_inv = np.ascontiguousarray(
        cnt_inv.reshape(d.n_graphs // P, P).T).astype(np.float32)
    return plan


def build_nc(plan: Plan):
    d = plan.dims
    SH, NT, WS, NG = d.shard, d.nt, d.ws, d.n_graphs
    KIN, HID, OUT = d.kin, d.hid, d.out
    NK = KIN // P
    NGT = NG // P
    K, chunk_base = plan.K, plan.chunk_base
    icols_total = int(plan.icols.sum())

    nc = bacc.Bacc("TRN2", target_bir_lowering=False, debug=False)

    x_t = nc.dram_tensor("x_t", [KIN, SH], BF16, kind="ExternalInput")
    eidx = nc.dram_tensor("eidx", [P, icols_total], I16, kind="ExternalInput")
    eld = nc.dram_tensor("eld", [P, plan.nch], F32, kind="ExternalInput")
    dinv_in = nc.dram_tensor("dinv", [P, NT], F32, kind="ExternalInput")
    bq_in = nc.dram_tensor("bq", [1, SH], BF16, kind="ExternalInput")
    bval_in = nc.dram_tensor("bval", [P, NT], F32, kind="ExternalInput")
    w1_in = nc.dram_tensor("w1", [KIN, HID], BF16, kind="ExternalInput")
    w2_in = nc.dram_tensor("w2", [HID, HID], BF16, kind="ExternalInput")
    b1_in = nc.dram_tensor("b1", [1, HID], BF16, kind="ExternalInput")
    b2_in = nc.dram_tensor("b2", [1, HID], BF16, kind="ExternalInput")
    fcw_in = nc.dram_tensor("fcw", [HID, OUT], F32, kind="ExternalInput")
    fcbt_in = nc.dram_tensor("fcbt", [P, OUT], F32, kind="ExternalInput")
    iota128_in = nc.dram_tensor("iota128", [P, P], BF16, kind="ExternalInput")
    iotag_in = nc.dram_tensor("iotag", [P, NG], F32, kind="ExternalInput")
    ident_in = nc.dram_tensor("ident", [P, P], BF16, kind="ExternalInput")
    cntinv_in = nc.dram_tensor("cntinv", [P, NGT], F32, kind="ExternalInput")
    out_t = nc.dram_tensor("out", [NG, OUT], F32, kind="ExternalOutput")

    with tile.TileContext(nc) as tc:
        with tc.tile_pool(name="const", bufs=1) as cp, \
             tc.tile_pool(name="xio", bufs=3) as xio, \
             tc.tile_pool(name="stage", bufs=3) as stp, \
             tc.tile_pool(name="gpool", bufs=2) as gp, \
             tc.tile_pool(name="spool", bufs=12) as sp, \
             tc.tile_pool(name="psA", bufs=3, space="PSUM") as psA, \
             tc.tile_pool(name="psB", bufs=2, space="PSUM") as psB, \
             tc.tile_pool(name="psC", bufs=2, space="PSUM") as psC, \
             tc.tile_pool(name="psPool", bufs=1, space="PSUM") as psP, \
             tc.tile_pool(name="dram", bufs=1, space="DRAM") as dp:

            # ---- constants ----
            w1_sb = cp.tile([P, NK, HID], BF16)
            nc.sync.dma_start(out=w1_sb[:], in_=w1_in.rearrange(
                "(k p) h -> p k h", p=P))
            w2_sb = cp.tile([P, HID], BF16)
            nc.sync.dma_start(out=w2_sb[:], in_=w2_in[:])
            b1_sb = cp.tile([1, HID], BF16)
            nc.sync.dma_start(out=b1_sb[:], in_=b1_in[:])
            b2_sb = cp.tile([1, HID], BF16)
            nc.sync.dma_start(out=b2_sb[:], in_=b2_in[:])
            bq_sb = cp.tile([1, SH], BF16)
            nc.sync.dma_start(out=bq_sb[:], in_=bq_in[:])
            dinv_sb = cp.tile([P, NT], F32)
            nc.sync.dma_start(out=dinv_sb[:], in_=dinv_in[:])
            bval_sb = cp.tile([P, NT], F32)
            nc.sync.dma_start(out=bval_sb[:], in_=bval_in[:])
            iota_sb = cp.tile([P, P], BF16)
            nc.sync.dma_start(out=iota_sb[:], in_=iota128_in[:])
            iotag_sb = cp.tile([P, NG], F32)
            nc.sync.dma_start(out=iotag_sb[:], in_=iotag_in[:])
            ident_sb = cp.tile([P, P], BF16)
            nc.sync.dma_start(out=ident_sb[:], in_=ident_in[:])
            fcw_sb = cp.tile([P, OUT], F32)
            nc.sync.dma_start(out=fcw_sb[:], in_=fcw_in[:])
            fcbt_sb = cp.tile([P, OUT], F32)
            nc.sync.dma_start(out=fcbt_sb[:], in_=fcbt_in[:])
            cntinv_sb = cp.tile([P, NGT], F32)
            nc.sync.dma_start(out=cntinv_sb[:], in_=cntinv_in[:])
            eld_sb = cp.tile([P, plan.nch], F32)
            nc.sync.dma_start(out=eld_sb[:], in_=eld[:])
            pkeep1 = cp.tile([P, NT * P], BF16)
            pkeep2 = cp.tile([P, NT * P], BF16)

            shard1 = dp.tile([SH, HID], BF16)
            shard2 = dp.tile([SH, HID], BF16)
            table1 = dp.tile([d.npad, HID], BF16, addr_space="Shared")
            table2 = dp.tile([d.npad, HID], BF16, addr_space="Shared")
            poolpart = dp.tile([P, NG], F32)
            poolsum = dp.tile([P, NG], F32, addr_space="Shared")

            # ---- stage 1: p1' = dinv * (x @ W1), write shard1 ----
            xr = x_t.rearrange("(k p) n -> p k n", p=P)
            for t in range(NT):
                xt = xio.tile([P, NK, P], BF16, tag="xt")
                nc.sync.dma_start(out=xt[:], in_=xr[:, :, t * P:(t + 1) * P])
                ps = psA.tile([P, HID], F32, space="PSUM", tag="acc")
                for k in range(NK):
                    nc.tensor.matmul(ps[:], xt[:, k, :], w1_sb[:, k, :],
                                     start=(k == 0), stop=(k == NK - 1))
                nc.scalar.activation(out=pkeep1[:, t * P:(t + 1) * P],
                                     in_=ps[:],
                                     func=mybir.ActivationFunctionType.Copy,
                                     scale=dinv_sb[:, t:t + 1])
                nc.sync.dma_start(out=shard1[t * P:(t + 1) * P, :],
                                  in_=pkeep1[:, t * P:(t + 1) * P])

            nc.gpsimd.collective_compute(
                "AllGather", mybir.AluOpType.bypass,
                replica_groups=[list(range(NCORES))],
                ins=[shard1.opt()], outs=[table1.opt()])

            # ---- conv aggregation stage (used twice) ----
            def conv_stage(table, b_sb, pk, is_conv2):
                for si, ts in enumerate(plan.st_tiles):
                    gts = []
                    for w in range(NWIN):
                        nchunks = int(K[ts, w].sum())
                        if nchunks == 0:
                            gts.append(None)
                            continue
                        ic = int(plan.icols[si, w])
                        ib = int(plan.icol_base[si, w])
                        idxs = sp.tile([P, ic], I16, tag="idx")
                        nc.sync.dma_start(out=idxs[:], in_=eidx[:, ib:ib + ic])
                        g = gp.tile([P, nchunks, HID], BF16, tag=f"g{w}")
                        # one call per (t, w) with num_idxs = real max count
                        # (trailing slots unfetched; S rows there are zero),
                        # split at the 1024-idx HW cap
                        cbase = 0
                        for t in ts:
                            cnt = int(plan.maxcnt[t, w])
                            if cnt == 0:
                                continue
                            coff = int(plan.tw_col_off[t, w] - ib)
                            done = 0
                            while done < cnt:
                                nidx = min(1024, cnt - done)
                                nidx = ((nidx + 15) // 16) * 16  # HW: x16
                                nchk = (nidx + P - 1) // P
                                icw = nidx // 16
                                nc.gpsimd.dma_gather(
                                    g[:, cbase + done // P:
                                      cbase + done // P + nchk, :],
                                    table[w * WS:(w + 1) * WS, :],
                                    idxs[:, coff + done // 16:
                                         coff + done // 16 + icw],
                                    nidx, nidx, HID)
                                done += nidx
                            cbase += int(K[t, w])
                        gts.append(g)

                    for t in ts:
                        ps = psA.tile([P, HID], F32, space="PSUM", tag="acc")
                        # self-loop term: local pre-scaled rows, no gather
                        nc.tensor.matmul(ps[:], ident_sb[:],
                                         pk[:, t * P:(t + 1) * P],
                                         start=True, stop=False)
                        first = False
                        for w in range(NWIN):
                            kw = int(K[t, w])
                            if kw == 0:
                                continue
                            # chunk offset of tile t within g tile of window w
                            base_in_g = sum(int(K[tt, w]) for tt in ts if tt < t)
                            cb = int(chunk_base[t, w])
                            g = gts[w]
                            for ci in range(kw):
                                s = sp.tile([P, P], BF16, tag="s")
                                col = cb + ci
                                nc.vector.tensor_scalar(
                                    out=s[:], in0=iota_sb[:],
                                    scalar1=eld_sb[:, col:col + 1], scalar2=None,
                                    op0=mybir.AluOpType.is_equal)
                                nc.tensor.matmul(
                                    ps[:], s[:], g[:, base_in_g + ci, :],
                                    start=False, stop=False)
                        # bias as rank-1 outer(sqrt(deg), b): after the
                        # dinv post-scale this contributes exactly +b
                        nc.tensor.matmul(
                            ps[:], bq_sb[0:1, t * P:(t + 1) * P], b_sb[0:1, :],
                            start=False, stop=True)

                        h = stp.tile([P, HID], BF16, tag="h")
                        nc.scalar.activation(
                            out=h[:], in_=ps[:],
                            func=mybir.ActivationFunctionType.Relu,
                            scale=dinv_sb[:, t:t + 1])

                        if not is_conv2:
                            # h1 -> h1T -> p2' = dinv * (h1 @ W2) -> shard2
                            pst = psB.tile([P, HID], BF16, space="PSUM", tag="tmp")
                            nc.tensor.transpose(pst[:], h[:], ident_sb[:])
                            ht = stp.tile([P, HID], BF16, tag="ht")
                            nc.vector.tensor_copy(out=ht[:], in_=pst[:])
                            ps2 = psC.tile([P, HID], F32, space="PSUM", tag="p2")
                            nc.tensor.matmul(ps2[:], ht[:], w2_sb[:],
                                             start=True, stop=True)
                            nc.scalar.activation(
                                out=pkeep2[:, t * P:(t + 1) * P], in_=ps2[:],
                                func=mybir.ActivationFunctionType.Copy,
                                scale=dinv_sb[:, t:t + 1])
                            nc.sync.dma_start(
                                out=shard2[t * P:(t + 1) * P, :],
                                in_=pkeep2[:, t * P:(t + 1) * P])
                        else:
                            # mean-pool partial sums: poolT[f, g] += h2^T @ 1hot
                            spool = sp.tile([P, NG], BF16, tag="spool")
                            nc.vector.tensor_scalar(
                                out=spool[:], in0=iotag_sb[:],
                                scalar1=bval_sb[:, t:t + 1], scalar2=None,
                                op0=mybir.AluOpType.is_equal)
                            nc.tensor.matmul(
                                pool_ps[:], h[:], spool[:],
                                start=(t == 0), stop=(t == NT - 1))

            # one-time zero of the G pool slots: trailing unfetched slots
            # of trimmed gathers must hold finite values (0 * NaN = NaN)
            for w in range(NWIN):
                gmax = max(int(K[ts, w].sum()) for ts in plan.st_tiles)
                if gmax == 0:
                    continue
                for b in range(2):
                    gz = gp.tile([P, gmax, HID], BF16, tag=f"g{w}",
                                 name=f"gz{w}_{b}")
                    nc.vector.memset(gz[:], 0)

            conv_stage(table1, b1_sb, pkeep1, False)

            nc.gpsimd.collective_compute(
                "AllGather", mybir.AluOpType.bypass,
                replica_groups=[list(range(NCORES))],
                ins=[shard2.opt()], outs=[table2.opt()])

            pool_ps = psP.tile([P, NG], F32, space="PSUM")
            conv_stage(table2, b2_sb, pkeep2, True)

            # ---- pool partials -> AllReduce ----
            poolsb = stp.tile([P, NG], F32, tag="poolsb")
            nc.scalar.activation(out=poolsb[:], in_=pool_ps[:],
                                 func=mybir.ActivationFunctionType.Copy)
            nc.sync.dma_start(out=poolpart[:], in_=poolsb[:])
            nc.gpsimd.collective_compute(
                "AllReduce", mybir.AluOpType.add,
                replica_groups=[list(range(NCORES))],
                ins=[poolpart.opt()], outs=[poolsum.opt()])
            sums = cp.tile([P, NG], F32)
            nc.sync.dma_start(out=sums[:], in_=poolsum[:])

            # ---- head: logits + log_softmax ----
            for gt in range(NGT):
                psl = psB.tile([P, OUT], F32, space="PSUM", tag="tmp")
                nc.tensor.matmul(psl[:], sums[:, gt * P:(gt + 1) * P],
                                 fcw_sb[:], start=True, stop=True)
                lg = stp.tile([P, OUT], F32, tag="lg")
                nc.scalar.activation(out=lg[:], in_=psl[:],
                                     func=mybir.ActivationFunctionType.Copy,
                                     scale=cntinv_sb[:, gt:gt + 1])
                lgb = stp.tile([P, OUT], F32, tag="lgb")
                nc.vector.tensor_tensor(out=lgb[:], in0=lg[:], in1=fcbt_sb[:],
                                        op=mybir.AluOpType.add)
                mx = stp.tile([P, 1], F32, tag="mx")
                nc.vector.reduce_max(mx[:], lgb[:], axis=mybir.AxisListType.X)
                zc = stp.tile([P, OUT], F32, tag="zc")
                nc.vector.tensor_scalar(out=zc[:], in0=lgb[:], scalar1=mx[:, 0:1],
                                        scalar2=None,
                                        op0=mybir.AluOpType.subtract)
                ex = stp.tile([P, OUT], F32, tag="ex")
                nc.scalar.activation(out=ex[:], in_=zc[:],
                                     func=mybir.ActivationFunctionType.Exp)
                sm = stp.tile([P, 1], F32, tag="sm")
                nc.vector.reduce_sum(sm[:], ex[:], axis=mybir.AxisListType.X)
                ls = stp.tile([P, 1], F32, tag="ls")
                nc.scalar.activation(out=ls[:], in_=sm[:],
                                     func=mybir.ActivationFunctionType.Ln)
                res = stp.tile([P, OUT], F32, tag="res")
                nc.vector.tensor_scalar(out=res[:], in0=zc[:], scalar1=ls[:, 0:1],
                                        scalar2=None,
                                        op0=mybir.AluOpType.subtract)
                nc.sync.dma_start(out=out_t[gt * P:(gt + 1) * P, :], in_=res[:])

    nc.compile()
    return nc


def make_in_maps(plan: Plan, x, W1, b1, W2, b2, fc_w, fc_b):
    d = plan.dims
    SR, SH, NG = d.shard_real, d.shard, d.n_graphs
    bf = ml_dtypes.bfloat16

    x = np.asarray(x, np.float32)
    w1 = np.asarray(W1, np.float32).astype(bf)
    w2 = np.asarray(W2, np.float32).astype(bf)
    b1a = np.asarray(b1, np.float32).astype(bf).reshape(1, -1)
    b2a = np.asarray(b2, np.float32).astype(bf).reshape(1, -1)
    fcw = np.ascontiguousarray(np.asarray(fc_w, np.float32))
    fcbt = np.tile(np.asarray(fc_b, np.float32).reshape(1, -1), (P, 1))
    iota128 = np.tile(np.arange(P, dtype=np.float32)[None, :], (P, 1)).astype(bf)
    iotag = np.tile(np.arange(NG, dtype=np.float32)[None, :], (P, 1))
    ident = np.eye(P, dtype=np.float32).astype(bf)

    in_maps = []
    for c in range(NCORES):
        xs = np.zeros((SH, d.kin), np.float32)
        xs[:SR] = x[c * SR:(c + 1) * SR]
        in_maps.append({
            "x_t": np.ascontiguousarray(xs.T).astype(bf),
            "eidx": plan.eidx[c],
            "eld": plan.eld[c],
            "dinv": plan.dinv2d[c],
            "bq": plan.bq[c],
            "bval": plan.batchval[c],
            "w1": w1, "w2": w2, "b1": b1a, "b2": b2a,
            "fcw": fcw, "fcbt": fcbt,
            "iota128": iota128, "iotag": iotag, "ident": ident,
            "cntinv": plan.cnt_inv,
        })
    return in_maps


def prepare(x, edge_index, batch, W1, b1, W2, b2, fc_w, fc_b, dims=None):
    d = dims or Dims(n_nodes=x.shape[0], n_graphs=512, kin=x.shape[1],
                     hid=W1.shape[1], out=fc_w.shape[1])
    plan = make_plan(np.asarray(edge_index), np.asarray(batch), d)
    nc = build_nc(plan)
    in_maps = make_in_maps(plan, x, W1, b1, W2, b2, fc_w, fc_b)
    return nc, in_maps, plan


def kernel(x, edge_index, batch, W1, b1, W2, b2, fc_w, fc_b):
    nc, in_maps, _ = prepare(np.asarray(x), edge_index, batch,
                             W1, b1, W2, b2, fc_w, fc_b)
    res = run_bass_kernel_spmd(nc, in_maps, list(range(NCORES)))
    return np.ascontiguousarray(res.results[0]["out"].astype(np.float32))
